# revision 1
# baseline (speedup 1.0000x reference)
"""Conditional BatchNorm1d (training mode) on 8 Trainium2 NeuronCores.

Strategy (data-parallel over N):
  - Shard x/labels along N across 8 cores (62500 rows each).
  - One-hot encodings of labels (both layouts) are precomputed host-side in
    bf16 and streamed in (~4 MB/core extra traffic; frees DVE/GPSIMD, whose
    16-partition one-hot builds dominated earlier profiles).
  - Pass 1 (per core): segment sums s1[c,f] = sum_{i: lab=c} x, s2 = sum x^2
    via one-hot matmul on the PE accumulating into PSUM. x is cast to bf16
    during the SWDGE DMA (halves pass-1 HBM traffic; the bf16 rounding error
    cancels statistically in the 31k-sample sums).
  - AllReduce the tiny [16,256] stats across the 8 cores.
  - Stats -> scale/shift [16,256] on-chip (mirrors the reference formulas).
  - Pass 2 (per core): per-row gather of scale/shift via transposed one-hot
    matmul in bf16 with hi/lo split (PSUM accumulation adds the halves, so
    the gather is fp32-exact to ~1e-7), then y = x*s + t on the DVE with
    quad-packed 3-D-AP ops.

Everything is hardcoded for the problem size: x [500000,128] f32,
labels [500000] int, gamma/beta [16,128] f32.
"""
import numpy as np

N_CORES = 8
N = 500000
F = 128
C = 16
EPS = 1e-5

SHARD = N // N_CORES         # 62500 real rows per core
P = 128                      # partitions per tile (16 DMA descriptors/transfer)
J = 20                       # subtiles per group (rows per partition)
GROUP = P * J                # 2560 rows per group
NG = 25                      # groups per core
ROWS = NG * GROUP            # 64000 padded rows per core
QUAD = 4                     # j-subtiles per psum tile / DVE op

_CACHE = {}


def _build():
    import concourse.bacc as bacc
    import concourse.bass as bass
    from concourse import mybir
    import concourse.tile as tile

    F32 = mybir.dt.float32
    BF16 = mybir.dt.bfloat16
    AF = mybir.ActivationFunctionType
    ALU = mybir.AluOpType

    nc = bacc.Bacc("TRN2", target_bir_lowering=False, debug=False,
                   num_devices=N_CORES)
    x = nc.dram_tensor("x", [ROWS, F], F32, kind="ExternalInput").ap()
    xb = nc.dram_tensor("xb", [ROWS, F], BF16, kind="ExternalInput").ap()
    h1 = nc.dram_tensor("h1", [ROWS, C], BF16, kind="ExternalInput").ap()
    ht = nc.dram_tensor("ht", [C, ROWS], BF16, kind="ExternalInput").ap()
    gamma = nc.dram_tensor("gamma", [C, F], F32, kind="ExternalInput").ap()
    beta = nc.dram_tensor("beta", [C, F], F32, kind="ExternalInput").ap()
    invn = nc.dram_tensor("invn", [C, 1], F32, kind="ExternalInput").ap()
    y = nc.dram_tensor("y", [ROWS, F], F32, kind="ExternalOutput").ap()

    with tile.TileContext(nc) as tc:
        with (
            tc.tile_pool(name="const", bufs=1) as const,
            tc.tile_pool(name="small", bufs=1) as small,
            tc.tile_pool(name="dram", bufs=1, space="DRAM") as dram,
            tc.tile_pool(name="psacc", bufs=1, space="PSUM") as psacc,
        ):
            # ---- constants ----
            gamma_sb = const.tile([C, F], F32)
            nc.sync.dma_start(out=gamma_sb[:], in_=gamma)
            beta_sb = const.tile([C, F], F32)
            nc.sync.dma_start(out=beta_sb[:], in_=beta)
            invn_sb = const.tile([C, 1], F32)
            nc.sync.dma_start(out=invn_sb[:], in_=invn)
            eps_sb = const.tile([C, 1], F32)
            nc.vector.memset(eps_sb[:], EPS)

            # ================= PASS 1: local stats =================
            psum_s12 = psacc.tile([C, 2 * F], F32)
            with tc.tile_pool(name="p1", bufs=4) as p1:
                for g in range(NG):
                    base = g * GROUP
                    # p-major: partition p holds rows [base+J*p, base+J*(p+1))
                    x_p = bass.AP(tensor=xb.tensor, offset=base * F,
                                  ap=[[J * F, P], [1, J * F]])
                    # xc = [x (J*F) | x^2 (J*F)]: both halves contiguous;
                    # matmul rhs reads [x_j | xsq_j] via a 2-D free AP.
                    xc = p1.tile([P, 2, J * F], BF16)
                    nc.sync.dma_start(out=xc[:, 0, :].opt(), in_=x_p.opt())
                    if g % 2 == 0:
                        nc.scalar.activation(out=xc[:, 1, :].opt(),
                                             in_=xc[:, 0, :].opt(), func=AF.Square)
                    else:
                        nc.vector.tensor_tensor(out=xc[:, 1, :].opt(),
                                                in0=xc[:, 0, :].opt(),
                                                in1=xc[:, 0, :].opt(),
                                                op=ALU.mult)
                    # one-hot H [125, 20, 16] (host-precomputed, contiguous)
                    h_p = bass.AP(tensor=h1.tensor, offset=base * C,
                                  ap=[[J * C, P], [1, J * C]])
                    H = p1.tile([P, J, C], BF16, tag="H")
                    nc.sync.dma_start(out=H[:].opt(), in_=h_p.opt())

                    xc0 = xc[:].opt()
                    for j in range(J):
                        rhs_j = bass.AP(tensor=xc.tensor,
                                        offset=xc0.offset + j * F,
                                        ap=[xc0.ap[0], [J * F, 2], [1, F]])
                        nc.tensor.matmul(
                            out=psum_s12[:],
                            lhsT=H[:, j, :],
                            rhs=rhs_j,
                            start=(g == 0 and j == 0),
                            stop=(g == NG - 1 and j == J - 1),
                        )

            # ================= AllReduce stats =================
            stats_sb = small.tile([C, 2 * F], F32)
            nc.vector.tensor_copy(out=stats_sb[:], in_=psum_s12[:])
            cc_in = dram.tile([C, 2 * F], F32)
            cc_out = dram.tile([C, 2 * F], F32)
            nc.scalar.dma_start(out=cc_in[:], in_=stats_sb[:])
            nc.gpsimd.collective_compute(
                "AllReduce",
                mybir.AluOpType.add,
                replica_groups=[list(range(N_CORES))],
                ins=[cc_in.opt()],
                outs=[cc_out.opt()],
            )
            stats_all = small.tile([C, 2 * F], F32)
            nc.scalar.dma_start(out=stats_all[:], in_=cc_out[:])

            # ---- stats -> scale/shift (mirrors reference formulas) ----
            mean = small.tile([C, F], F32)
            nc.vector.tensor_scalar(out=mean[:], in0=stats_all[:, 0:F],
                                    scalar1=invn_sb[:], scalar2=None, op0=ALU.mult)
            ex2 = small.tile([C, F], F32)
            nc.vector.tensor_scalar(out=ex2[:], in0=stats_all[:, F:2 * F],
                                    scalar1=invn_sb[:], scalar2=None, op0=ALU.mult)
            var = small.tile([C, F], F32)
            nc.vector.tensor_tensor(out=var[:], in0=mean[:], in1=mean[:], op=ALU.mult)
            nc.vector.tensor_tensor(out=var[:], in0=ex2[:], in1=var[:], op=ALU.subtract)
            std = small.tile([C, F], F32)
            nc.scalar.activation(out=std[:], in_=var[:], func=AF.Sqrt, bias=eps_sb[:])
            istd = small.tile([C, F], F32)
            nc.vector.reciprocal(out=istd[:], in_=std[:])
            sc_sh = small.tile([C, 2 * F], F32)
            nc.vector.tensor_tensor(out=sc_sh[:, 0:F], in0=gamma_sb[:],
                                    in1=istd[:], op=ALU.mult)
            ms = small.tile([C, F], F32)
            nc.vector.tensor_tensor(out=ms[:], in0=mean[:], in1=sc_sh[:, 0:F],
                                    op=ALU.mult)
            nc.vector.tensor_tensor(out=sc_sh[:, F:2 * F], in0=beta_sb[:],
                                    in1=ms[:], op=ALU.subtract)
            # bf16 hi/lo split: hi + lo == sc_sh to ~1e-7 (PSUM adds them)
            sc_hi = small.tile([C, 2 * F], BF16)
            nc.vector.tensor_copy(out=sc_hi[:], in_=sc_sh[:])
            sc_lo = small.tile([C, 2 * F], BF16)
            nc.vector.tensor_tensor(out=sc_lo[:], in0=sc_sh[:], in1=sc_hi[:],
                                    op=ALU.subtract)

            # ================= PASS 2: apply =================
            # p-major x/y; ht columns are host-permuted to (g, j, p) order so
            # lhsT for subtile j is the contiguous slice ht[:, base+125j:...].
            with tc.tile_pool(name="p2", bufs=4) as p2, \
                 tc.tile_pool(name="p2y", bufs=3) as p2y, \
                 tc.tile_pool(name="p2t", bufs=4) as p2t, \
                 tc.tile_pool(name="ps2", bufs=3, space="PSUM") as ps2:
                for g in range(NG):
                    base = g * GROUP
                    x_p = bass.AP(tensor=x.tensor, offset=base * F,
                                  ap=[[J * F, P], [1, J * F]])
                    y_p = bass.AP(tensor=y.tensor, offset=base * F,
                                  ap=[[J * F, P], [1, J * F]])
                    x2_tile = p2.tile([P, J, F], F32)
                    nc.sync.dma_start(out=x2_tile[:].opt(), in_=x_p.opt())
                    ht_ap = bass.AP(tensor=ht.tensor, offset=base,
                                    ap=[[ROWS, C], [1, GROUP]])
                    H_T = p2.tile([C, GROUP], BF16, tag="HT")
                    nc.sync.dma_start(out=H_T[:].opt(), in_=ht_ap.opt())

                    y_tile = p2y.tile([P, J, F], F32)
                    for q in range(J // QUAD):
                        psum_ss = ps2.tile([P, QUAD, 2 * F], F32)  # 2 banks
                        for h in range(QUAD):
                            j = QUAD * q + h
                            lhsT_j = H_T[:, P * j:P * (j + 1)]
                            nc.tensor.matmul(out=psum_ss[:, h, :], lhsT=lhsT_j,
                                             rhs=sc_hi[:], start=True, stop=False)
                            nc.tensor.matmul(out=psum_ss[:, h, :], lhsT=lhsT_j,
                                             rhs=sc_lo[:], start=False, stop=True)
                        j0 = QUAD * q
                        tmp = p2t.tile([P, QUAD, F], F32)
                        nc.vector.tensor_tensor(out=tmp[:],
                                                in0=x2_tile[:, j0:j0 + QUAD, :],
                                                in1=psum_ss[:, :, 0:F],
                                                op=ALU.mult)
                        nc.vector.tensor_tensor(out=y_tile[:, j0:j0 + QUAD, :],
                                                in0=tmp[:],
                                                in1=psum_ss[:, :, F:2 * F],
                                                op=ALU.add)
                    nc.scalar.dma_start(out=y_p.opt(), in_=y_tile[:].opt())
    nc.finalize()
    return nc


def _get_nc():
    if "nc" not in _CACHE:
        _CACHE["nc"] = _build()
    return _CACHE["nc"]


def _prep_host(labels_np):
    import ml_dtypes
    BF = ml_dtypes.bfloat16
    lab = labels_np.astype(np.int64)
    counts = np.maximum(np.bincount(lab, minlength=C), 1).astype(np.float64)
    invn = (1.0 / counts).astype(np.float32).reshape(C, 1)
    eye = np.eye(C, dtype=BF)
    h1_all, ht_all = [], []
    for k in range(N_CORES):
        lab_pad = np.full(ROWS, -1, dtype=np.int64)
        lab_pad[:SHARD] = lab[k * SHARD:(k + 1) * SHARD]
        h1 = np.zeros((ROWS, C), dtype=BF)
        h1[:SHARD] = eye[lab_pad[:SHARD]]
        h1_all.append(h1)
        # ht columns in (g, j, p) order: col g*GROUP+P*j+p holds onehot of
        # padded row g*GROUP + J*p + j (zero for pad rows).
        shard = lab_pad.reshape(NG, P, J)                        # (g, p, j)
        perm = shard.transpose(0, 2, 1).reshape(-1)              # (g, j, p)
        onehot_t = (perm[None, :] == np.arange(C)[:, None])
        ht_all.append(onehot_t.astype(BF))
    return h1_all, ht_all, invn


def kernel(x, labels, gamma, beta):
    from concourse.bass_utils import run_bass_kernel_spmd

    x = np.ascontiguousarray(np.asarray(x, dtype=np.float32))
    labels_np = np.asarray(labels)
    gamma = np.ascontiguousarray(np.asarray(gamma, dtype=np.float32))
    beta = np.ascontiguousarray(np.asarray(beta, dtype=np.float32))

    h1_all, ht_all, invn = _prep_host(labels_np)
    import ml_dtypes
    xb = x.astype(ml_dtypes.bfloat16)

    nc = _get_nc()
    in_maps = []
    for k in range(N_CORES):
        sl = slice(k * SHARD, (k + 1) * SHARD)
        x_pad = np.zeros((ROWS, F), dtype=np.float32)
        x_pad[:SHARD] = x[sl]
        xb_pad = np.zeros((ROWS, F), dtype=ml_dtypes.bfloat16)
        xb_pad[:SHARD] = xb[sl]
        in_maps.append({
            "x": x_pad,
            "xb": xb_pad,
            "h1": h1_all[k],
            "ht": ht_all[k],
            "gamma": gamma,
            "beta": beta,
            "invn": invn,
        })
    res = run_bass_kernel_spmd(nc, in_maps, core_ids=list(range(N_CORES)),
                               **_CACHE.get("run_kwargs", {}))
    _CACHE["last_results"] = res
    y = np.concatenate([res.results[k]["y"][:SHARD] for k in range(N_CORES)],
                       axis=0)
    return y



# revision 3
# speedup vs baseline: 2.0422x; 2.0422x over previous
"""Conditional BatchNorm1d (training mode) on 8 Trainium2 NeuronCores.

Strategy (feature-parallel, sort-packed, zero collectives):
  - Host: stable-sort rows by label; split each label's rows evenly over 8
    row-chunks; pack a per-core [128, 16*CAP] bf16 array where partition
    p = chunk*16 + i holds feature (16*core + i) of that chunk's rows,
    one fixed-capacity zero-padded column slot per label.
  - Each core owns 16 of the 128 features for ALL 500k rows, so the global
    per-label statistics are computed entirely locally -- no AllReduce.
  - Pass 1 (pipelined behind the 16 slot DMAs): per label slot,
    sum(x) on DVE (tensor_scalar copy + accum_out) and sum(x^2) on the
    scalar engine (Square + accum_out) / DVE (scalar_tensor_tensor).
  - One tiny fp32 PE matmul folds the 8 chunk partials per feature,
    scale/shift are computed on 16 partitions (mirroring the reference
    formulas), and a second tiny matmul broadcasts them back to all 128
    partitions.
  - Pass 2: per label slot, y = x*scale + shift as a single per-partition
    tensor_scalar (DVE) or activation Identity (scalar engine), then DMA
    out. x stays SBUF-resident between the passes.
  - HBM traffic: 16 MB in + 16 MB out per core (bf16), ~2.7x less than a
    row-sharded fp32 formulation, and the sole serial dependency is
    read-all -> stats -> write-all.

Everything hardcoded for: x [500000,128] f32, labels [500000] int,
gamma/beta [16,128] f32, 16 conditions. bf16 data path keeps the norm
relative error ~2e-3, well inside the 2e-2 gate.
"""
import numpy as np

N_CORES = 8
N = 500000
F = 128
C = 16
EPS = 1e-5

FPC = F // N_CORES     # 16 features owned per core
NJ = 8                 # row-chunks -> partitions = NJ*FPC = 128
P = NJ * FPC           # 128
GRP = 512              # bn-free-dim granule; CAP is a multiple of this

# pass-1 sum(x^2) engine split: these label slots go to DVE, rest scalar
SQ_DVE = frozenset(range(10, 16))
# pass-2 apply engine split: these label slots go to the scalar engine
AP_SC = frozenset({2, 5, 8, 11, 14})

_CACHE = {}


def _build(cap):
    import concourse.bacc as bacc
    import concourse.bass as bass
    from concourse import mybir
    import concourse.tile as tile

    F32 = mybir.dt.float32
    BF16 = mybir.dt.bfloat16
    AF = mybir.ActivationFunctionType
    ALU = mybir.AluOpType
    cols = C * cap

    nc = bacc.Bacc("TRN2", target_bir_lowering=False, debug=False,
                   num_devices=N_CORES)
    xt = nc.dram_tensor("xt", [P, cols], BF16, kind="ExternalInput").ap()
    gbt = nc.dram_tensor("gbt", [FPC, 3 * C], F32, kind="ExternalInput").ap()
    sel = nc.dram_tensor("sel", [P, FPC], F32, kind="ExternalInput").ap()
    rep = nc.dram_tensor("rep", [FPC, P], F32, kind="ExternalInput").ap()
    y = nc.dram_tensor("y", [P, cols], BF16, kind="ExternalOutput").ap()

    def x_slice(s):
        return bass.AP(tensor=xt.tensor, offset=s * cap,
                       ap=[[cols, P], [1, cap]])

    def y_slice(s):
        return bass.AP(tensor=y.tensor, offset=s * cap,
                       ap=[[cols, P], [1, cap]])

    with tile.TileContext(nc) as tc:
        with (
            tc.tile_pool(name="const", bufs=1) as const,
            tc.tile_pool(name="xres", bufs=C) as xres,
            tc.tile_pool(name="sqv", bufs=2) as sqv,
            tc.tile_pool(name="sqs", bufs=2) as sqs,
            tc.tile_pool(name="ybuf", bufs=4) as ybuf,
            tc.tile_pool(name="ps", bufs=1, space="PSUM") as ps,
        ):
            # ---- constants (small, scalar ring so they land first) ----
            gbt_sb = const.tile([FPC, 3 * C], F32)
            nc.scalar.dma_start(out=gbt_sb[:], in_=gbt)
            sel_sb = const.tile([P, FPC], F32)
            nc.scalar.dma_start(out=sel_sb[:], in_=sel)
            rep_sb = const.tile([FPC, P], F32)
            nc.scalar.dma_start(out=rep_sb[:], in_=rep)
            eps_sb = const.tile([FPC, 1], F32)
            nc.vector.memset(eps_sb[:], EPS)

            # ---- resident x: one [P, cap] tile per label slot ----
            xts = []
            for s in range(C):
                t = xres.tile([P, cap], BF16, tag="x")
                eng = nc.sync if s % 2 == 0 else nc.scalar
                eng.dma_start(out=t[:], in_=x_slice(s))
                xts.append(t)

            # ---- pass 1: raw sums per (partition=(chunk,feat), slot) ----
            # s12 cols [0:C) = sum(x), [C:2C) = sum(x^2)
            s12 = const.tile([P, 2 * C], F32)
            for s in range(C):
                scr = sqv.tile([P, cap], BF16, tag="scrv")
                nc.vector.tensor_scalar(out=scr[:], in0=xts[s][:],
                                        scalar1=1.0, scalar2=0.0,
                                        op0=ALU.mult, op1=ALU.add,
                                        accum_out=s12[:, s:s + 1])
                if s in SQ_DVE:
                    scr2 = sqv.tile([P, cap], BF16, tag="scrv")
                    nc.vector.scalar_tensor_tensor(
                        out=scr2[:], in0=xts[s][:], scalar=1.0,
                        in1=xts[s][:], op0=ALU.mult, op1=ALU.mult,
                        accum_out=s12[:, C + s:C + s + 1])
                else:
                    scr2 = sqs.tile([P, cap], BF16, tag="scrs")
                    nc.scalar.activation(out=scr2[:], in_=xts[s][:],
                                         func=AF.Square,
                                         accum_out=s12[:, C + s:C + s + 1])

            # ---- fold the 8 chunk partials per feature (PE, fp32) ----
            psA = ps.tile([FPC, 2 * C], F32, tag="psA")
            nc.tensor.matmul(out=psA[:], lhsT=sel_sb[:], rhs=s12[:],
                             start=True, stop=True)
            sA = const.tile([FPC, 2 * C], F32)
            nc.vector.tensor_copy(out=sA[:], in_=psA[:])

            # ---- stats -> scale/shift on 16 partitions ----
            invn = gbt_sb[:, 2 * C:3 * C]
            meanc = const.tile([FPC, C], F32)
            nc.vector.tensor_tensor(out=meanc[:], in0=sA[:, 0:C],
                                    in1=invn, op=ALU.mult)
            ex2 = const.tile([FPC, C], F32)
            nc.vector.tensor_tensor(out=ex2[:], in0=sA[:, C:2 * C],
                                    in1=invn, op=ALU.mult)
            varc = const.tile([FPC, C], F32)
            nc.vector.tensor_tensor(out=varc[:], in0=meanc[:], in1=meanc[:],
                                    op=ALU.mult)
            nc.vector.tensor_tensor(out=varc[:], in0=ex2[:], in1=varc[:],
                                    op=ALU.subtract)
            stdc = const.tile([FPC, C], F32)
            nc.scalar.activation(out=stdc[:], in_=varc[:], func=AF.Sqrt,
                                 bias=eps_sb[:])
            istd = const.tile([FPC, C], F32)
            nc.vector.reciprocal(out=istd[:], in_=stdc[:])
            scsh = const.tile([FPC, 2 * C], F32)
            nc.vector.tensor_tensor(out=scsh[:, 0:C], in0=gbt_sb[:, 0:C],
                                    in1=istd[:], op=ALU.mult)
            msc = const.tile([FPC, C], F32)
            nc.vector.tensor_tensor(out=msc[:], in0=meanc[:],
                                    in1=scsh[:, 0:C], op=ALU.mult)
            nc.vector.tensor_tensor(out=scsh[:, C:2 * C], in0=gbt_sb[:, C:2 * C],
                                    in1=msc[:], op=ALU.subtract)

            # ---- broadcast scale/shift back to 128 partitions (PE) ----
            psB = ps.tile([P, 2 * C], F32, tag="psB")
            nc.tensor.matmul(out=psB[:], lhsT=rep_sb[:], rhs=scsh[:],
                             start=True, stop=True)
            ss = const.tile([P, 2 * C], F32)
            nc.vector.tensor_copy(out=ss[:], in_=psB[:])

            # ---- pass 2: y = x*scale + shift, slot by slot ----
            for s in range(C):
                yb = ybuf.tile([P, cap], BF16, tag="y")
                if s in AP_SC:
                    nc.scalar.activation(out=yb[:], in_=xts[s][:],
                                         func=AF.Identity,
                                         bias=ss[:, C + s:C + s + 1],
                                         scale=ss[:, s:s + 1])
                else:
                    nc.vector.tensor_scalar(out=yb[:], in0=xts[s][:],
                                            scalar1=ss[:, s:s + 1],
                                            scalar2=ss[:, C + s:C + s + 1],
                                            op0=ALU.mult, op1=ALU.add)
                eng = nc.sync if s % 2 == 0 else nc.scalar
                eng.dma_start(out=y_slice(s), in_=yb[:])
    nc.finalize()
    return nc


def _get_nc(cap):
    key = ("nc", cap)
    if key not in _CACHE:
        _CACHE[key] = _build(cap)
    return _CACHE[key]


def kernel(x, labels, gamma, beta):
    import ml_dtypes
    from concourse.bass_utils import run_bass_kernel_spmd

    BF = ml_dtypes.bfloat16
    x = np.asarray(x, dtype=np.float32)
    lab = np.asarray(labels).astype(np.int64).ravel()
    gamma = np.asarray(gamma, dtype=np.float32)
    beta = np.asarray(beta, dtype=np.float32)

    counts = np.bincount(lab, minlength=C).astype(np.int64)
    base, rem = counts // NJ, counts % NJ
    ncj = base[None, :] + (np.arange(NJ)[:, None] < rem[None, :])  # [NJ, C]
    cap = int(-(-int(ncj.max()) // GRP) * GRP)                     # pad to GRP
    cols = C * cap

    order = np.argsort(lab, kind="stable")
    starts = np.zeros(C + 1, np.int64)
    starts[1:] = np.cumsum(counts)
    # col_idx[j, c*cap + t] = original row index (N -> zero/garbage row)
    col_idx = np.full((NJ, cols), N, dtype=np.int64)
    for c in range(C):
        off = starts[c]
        for j in range(NJ):
            m = int(ncj[j, c])
            col_idx[j, c * cap:c * cap + m] = order[off:off + m]
            off += m

    xb = np.concatenate([x.astype(BF), np.zeros((1, F), BF)], axis=0)
    g = xb[col_idx.reshape(-1)]                    # [NJ*cols, F] bf16
    g = g.reshape(NJ, cols, F).transpose(0, 2, 1)  # [NJ, F, cols]

    invn = (1.0 / np.maximum(counts, 1)).astype(np.float32)
    gT, bT = gamma.T, beta.T                       # [F, C]
    selm = (np.arange(P)[:, None] % FPC == np.arange(FPC)[None, :])
    selm = selm.astype(np.float32)                 # [P, FPC]

    nc = _get_nc(cap)
    in_maps = []
    for k in range(N_CORES):
        f0 = k * FPC
        xk = np.ascontiguousarray(g[:, f0:f0 + FPC, :]).reshape(P, cols)
        gbk = np.concatenate(
            [gT[f0:f0 + FPC], bT[f0:f0 + FPC],
             np.broadcast_to(invn[None, :], (FPC, C))], axis=1,
        ).astype(np.float32)
        in_maps.append({
            "xt": xk,
            "gbt": np.ascontiguousarray(gbk),
            "sel": selm,
            "rep": np.ascontiguousarray(selm.T),
        })
    res = run_bass_kernel_spmd(nc, in_maps, core_ids=list(range(N_CORES)),
                               **_CACHE.get("run_kwargs", {}))
    _CACHE["last_results"] = res

    ys = np.empty((N + 1, F), dtype=np.float32)    # row N absorbs padding
    for k in range(N_CORES):
        f0 = k * FPC
        yk = np.asarray(res.results[k]["y"]).reshape(NJ, FPC, cols)
        yk = yk.transpose(0, 2, 1).astype(np.float32)  # [NJ, cols, FPC]
        for j in range(NJ):
            ys[col_idx[j], f0:f0 + FPC] = yk[j]
    return np.ascontiguousarray(ys[:N])


# revision 4
# speedup vs baseline: 2.5499x; 1.2486x over previous
"""Conditional BatchNorm1d (training mode) on 8 Trainium2 NeuronCores.

Strategy (feature-parallel, sort-packed, zero collectives):
  - Host: stable-sort rows by label; split each label's rows evenly over 8
    row-chunks; pack a per-core [128, 16*CAP] bf16 array where partition
    p = chunk*16 + i holds feature (16*core + i) of that chunk's rows,
    one fixed-capacity zero-padded column slot per label.
  - Each core owns 16 of the 128 features for ALL 500k rows, so the global
    per-label statistics are computed entirely locally -- no AllReduce.
  - Pass 1 (pipelined behind the 16 slot DMAs): per label slot, sum(x) via
    DVE tensor_reduce and sum(x^2) via scalar-engine Square+accum_out
    (two slots ride DVE's scalar_tensor_tensor to balance the engines).
  - One tiny fp32 PE matmul with a [128,128] selector-replicator both
    folds the 8 chunk partials per feature and broadcasts the result to
    all 128 partitions; scale/shift are then computed in-place on 128
    partitions (mirroring the reference formulas).
  - Pass 2: per label slot, y = x*scale + shift as a single per-partition
    tensor_scalar (DVE) or activation Identity (scalar engine), DMA out.
  - x stays SBUF-resident between the passes: HBM traffic is ~16 MB in +
    ~16 MB out per core (bf16). gpsimd is left idle on purpose: it shares
    an SBUF port with DVE (exclusive lock) and stalls it.

Everything hardcoded for: x [500000,128] f32, labels [500000] int,
gamma/beta [16,128] f32, 16 conditions. bf16 data path keeps the norm
relative error ~2e-3, well inside the 2e-2 gate.
"""
import numpy as np

N_CORES = 8
N = 500000
F = 128
C = 16
EPS = 1e-5

FPC = F // N_CORES     # 16 features owned per core
NJ = 8                 # row-chunks -> partitions = NJ*FPC = 128
P = NJ * FPC           # 128
ALIGN = 64             # CAP granularity (128B DMA lines)

# pass-1 sum(x^2): these label slots go to DVE (scalar_tensor_tensor),
# the rest to the scalar engine (Square + accum_out)
SQ_DVE = frozenset({1, 3})
# pass-2 apply: these label slots go to the scalar engine
AP_SC = frozenset({2, 6, 10, 14})

_CACHE = {}


def _build(cap):
    import concourse.bacc as bacc
    import concourse.bass as bass
    from concourse import mybir
    import concourse.tile as tile

    F32 = mybir.dt.float32
    BF16 = mybir.dt.bfloat16
    AF = mybir.ActivationFunctionType
    ALU = mybir.AluOpType
    cols = C * cap

    nc = bacc.Bacc("TRN2", target_bir_lowering=False, debug=False,
                   num_devices=N_CORES)
    xt = nc.dram_tensor("xt", [P, cols], BF16, kind="ExternalInput").ap()
    gbt = nc.dram_tensor("gbt", [P, 3 * C], F32, kind="ExternalInput").ap()
    selrep = nc.dram_tensor("selrep", [P, P], F32, kind="ExternalInput").ap()
    y = nc.dram_tensor("y", [P, cols], BF16, kind="ExternalOutput").ap()

    def x_slice(s):
        return bass.AP(tensor=xt.tensor, offset=s * cap,
                       ap=[[cols, P], [1, cap]])

    def y_slice(s):
        return bass.AP(tensor=y.tensor, offset=s * cap,
                       ap=[[cols, P], [1, cap]])

    with tile.TileContext(nc) as tc:
        with (
            tc.tile_pool(name="const", bufs=1) as const,
            tc.tile_pool(name="xres", bufs=C) as xres,
            tc.tile_pool(name="sqv", bufs=2) as sqv,
            tc.tile_pool(name="sqs", bufs=2) as sqs,
            tc.tile_pool(name="ybuf", bufs=4) as ybuf,
            tc.tile_pool(name="ps", bufs=1, space="PSUM") as ps,
        ):
            # ---- constants (small, scalar ring so they land first) ----
            gbt_sb = const.tile([P, 3 * C], F32)
            nc.scalar.dma_start(out=gbt_sb[:], in_=gbt)
            sel_sb = const.tile([P, P], F32)
            nc.scalar.dma_start(out=sel_sb[:], in_=selrep)
            eps_sb = const.tile([P, 1], F32)
            nc.vector.memset(eps_sb[:], EPS)

            # ---- resident x: one [P, cap] tile per label slot ----
            xts = []
            for s in range(C):
                t = xres.tile([P, cap], BF16, tag="x")
                eng = nc.sync if s % 2 == 0 else nc.scalar
                eng.dma_start(out=t[:], in_=x_slice(s))
                xts.append(t)

            # ---- pass 1: raw sums per (partition=(chunk,feat), slot) ----
            # s12 cols [0:C) = sum(x), [C:2C) = sum(x^2)
            s12 = const.tile([P, 2 * C], F32)
            for s in range(C):
                nc.vector.tensor_reduce(s12[:, s:s + 1], xts[s][:],
                                        axis=mybir.AxisListType.X,
                                        op=ALU.add)
                if s in SQ_DVE:
                    scr = sqv.tile([P, cap], BF16, tag="scrv")
                    nc.vector.scalar_tensor_tensor(
                        out=scr[:], in0=xts[s][:], scalar=1.0,
                        in1=xts[s][:], op0=ALU.mult, op1=ALU.mult,
                        accum_out=s12[:, C + s:C + s + 1])
                else:
                    scr = sqs.tile([P, cap], BF16, tag="scrs")
                    nc.scalar.activation(out=scr[:], in_=xts[s][:],
                                         func=AF.Square,
                                         accum_out=s12[:, C + s:C + s + 1])

            # ---- fold chunk partials per feature AND replicate to all
            #      128 partitions in one PE matmul (selrep[p,m]=[p==m mod 16])
            psA = ps.tile([P, 2 * C], F32, tag="psA")
            nc.tensor.matmul(out=psA[:], lhsT=sel_sb[:], rhs=s12[:],
                             start=True, stop=True)

            # ---- stats -> scale/shift on 128 partitions ----
            invn = gbt_sb[:, 2 * C:3 * C]
            meanc = const.tile([P, C], F32)
            nc.vector.tensor_tensor(out=meanc[:], in0=psA[:, 0:C],
                                    in1=invn, op=ALU.mult)
            ex2 = const.tile([P, C], F32)
            nc.vector.tensor_tensor(out=ex2[:], in0=psA[:, C:2 * C],
                                    in1=invn, op=ALU.mult)
            varc = const.tile([P, C], F32)
            nc.vector.tensor_tensor(out=varc[:], in0=meanc[:], in1=meanc[:],
                                    op=ALU.mult)
            nc.vector.tensor_tensor(out=varc[:], in0=ex2[:], in1=varc[:],
                                    op=ALU.subtract)
            stdc = const.tile([P, C], F32)
            nc.scalar.activation(out=stdc[:], in_=varc[:], func=AF.Sqrt,
                                 bias=eps_sb[:])
            istd = const.tile([P, C], F32)
            nc.vector.reciprocal(out=istd[:], in_=stdc[:])
            ss = const.tile([P, 2 * C], F32)
            nc.vector.tensor_tensor(out=ss[:, 0:C], in0=gbt_sb[:, 0:C],
                                    in1=istd[:], op=ALU.mult)
            msc = const.tile([P, C], F32)
            nc.vector.tensor_tensor(out=msc[:], in0=meanc[:],
                                    in1=ss[:, 0:C], op=ALU.mult)
            nc.vector.tensor_tensor(out=ss[:, C:2 * C], in0=gbt_sb[:, C:2 * C],
                                    in1=msc[:], op=ALU.subtract)

            # ---- pass 2: y = x*scale + shift, slot by slot ----
            for s in range(C):
                yb = ybuf.tile([P, cap], BF16, tag="y")
                if s in AP_SC:
                    nc.scalar.activation(out=yb[:], in_=xts[s][:],
                                         func=AF.Identity,
                                         bias=ss[:, C + s:C + s + 1],
                                         scale=ss[:, s:s + 1])
                else:
                    nc.vector.tensor_scalar(out=yb[:], in0=xts[s][:],
                                            scalar1=ss[:, s:s + 1],
                                            scalar2=ss[:, C + s:C + s + 1],
                                            op0=ALU.mult, op1=ALU.add)
                eng = nc.sync if s % 2 == 0 else nc.scalar
                eng.dma_start(out=y_slice(s), in_=yb[:])
    nc.finalize()
    return nc


def _get_nc(cap):
    key = ("nc", cap)
    if key not in _CACHE:
        _CACHE[key] = _build(cap)
    return _CACHE[key]


def kernel(x, labels, gamma, beta):
    import ml_dtypes
    from concourse.bass_utils import run_bass_kernel_spmd

    BF = ml_dtypes.bfloat16
    x = np.asarray(x, dtype=np.float32)
    lab = np.asarray(labels).astype(np.int64).ravel()
    gamma = np.asarray(gamma, dtype=np.float32)
    beta = np.asarray(beta, dtype=np.float32)

    counts = np.bincount(lab, minlength=C).astype(np.int64)
    base, rem = counts // NJ, counts % NJ
    ncj = base[None, :] + (np.arange(NJ)[:, None] < rem[None, :])  # [NJ, C]
    cap = int(-(-int(ncj.max()) // ALIGN) * ALIGN)
    cols = C * cap

    order = np.argsort(lab, kind="stable")
    starts = np.zeros(C + 1, np.int64)
    starts[1:] = np.cumsum(counts)
    # col_idx[j, c*cap + t] = original row index (N -> zero/garbage row)
    col_idx = np.full((NJ, cols), N, dtype=np.int64)
    for c in range(C):
        off = starts[c]
        for j in range(NJ):
            m = int(ncj[j, c])
            col_idx[j, c * cap:c * cap + m] = order[off:off + m]
            off += m

    xb = np.concatenate([x.astype(BF), np.zeros((1, F), BF)], axis=0)
    g = xb[col_idx.reshape(-1)]                    # [NJ*cols, F] bf16
    g = g.reshape(NJ, cols, F).transpose(0, 2, 1)  # [NJ, F, cols]

    invn = (1.0 / np.maximum(counts, 1)).astype(np.float32)
    gT, bT = gamma.T, beta.T                       # [F, C]
    selrep = (np.arange(P)[:, None] % FPC == np.arange(P)[None, :] % FPC)
    selrep = np.ascontiguousarray(selrep.astype(np.float32))

    nc = _get_nc(cap)
    in_maps = []
    for k in range(N_CORES):
        f0 = k * FPC
        xk = np.ascontiguousarray(g[:, f0:f0 + FPC, :]).reshape(P, cols)
        gbk = np.concatenate(
            [gT[f0:f0 + FPC], bT[f0:f0 + FPC],
             np.broadcast_to(invn[None, :], (FPC, C))], axis=1,
        ).astype(np.float32)
        gbk = np.ascontiguousarray(np.tile(gbk, (NJ, 1)))  # [P, 3C]
        in_maps.append({
            "xt": xk,
            "gbt": gbk,
            "selrep": selrep,
        })
    res = run_bass_kernel_spmd(nc, in_maps, core_ids=list(range(N_CORES)),
                               **_CACHE.get("run_kwargs", {}))
    _CACHE["last_results"] = res

    ys = np.empty((N + 1, F), dtype=np.float32)    # row N absorbs padding
    for k in range(N_CORES):
        f0 = k * FPC
        yk = np.asarray(res.results[k]["y"]).reshape(NJ, FPC, cols)
        yk = yk.transpose(0, 2, 1).astype(np.float32)  # [NJ, cols, FPC]
        for j in range(NJ):
            ys[col_idx[j], f0:f0 + FPC] = yk[j]
    return np.ascontiguousarray(ys[:N])


# revision 8
# speedup vs baseline: 2.7961x; 1.0965x over previous
"""Conditional BatchNorm1d (training mode) on 8 Trainium2 NeuronCores.

Strategy (feature-parallel, sort-packed, zero collectives):
  - Host: stable-sort rows by label; split each label's rows evenly over 8
    row-chunks; pack a per-core [128, 16*CAP] bf16 array where partition
    p = chunk*16 + i holds feature (16*core + i) of that chunk's rows,
    one fixed-capacity zero-padded column slot per label.
  - Each core owns 16 of the 128 features for ALL 500k rows, so the global
    per-label statistics are computed entirely locally -- no AllReduce.
  - Pass 1 (pipelined behind the 16 slot DMAs): per label slot, sum(x) via
    DVE tensor_reduce and sum(x^2) via scalar-engine Square+accum_out
    (two slots ride DVE's scalar_tensor_tensor to balance the engines).
  - One tiny fp32 PE matmul with a [128,128] selector-replicator both
    folds the 8 chunk partials per feature and broadcasts the result to
    all 128 partitions; scale/shift are then computed in-place on 128
    partitions (mirroring the reference formulas).
  - Pass 2: per label slot, y = x*scale + shift as a single per-partition
    tensor_scalar (DVE) or activation Identity (scalar engine), DMA out.
  - x stays SBUF-resident between the passes: HBM traffic is ~16 MB in +
    ~16 MB out per core (bf16). gpsimd is left idle on purpose: it shares
    an SBUF port with DVE (exclusive lock) and stalls it.

Everything hardcoded for: x [500000,128] f32, labels [500000] int,
gamma/beta [16,128] f32, 16 conditions. bf16 data path keeps the norm
relative error ~2e-3, well inside the 2e-2 gate.
"""
import numpy as np

N_CORES = 8
N = 500000
F = 128
C = 16
EPS = 1e-5

FPC = F // N_CORES     # 16 features owned per core
NJ = 8                 # row-chunks -> partitions = NJ*FPC = 128
P = NJ * FPC           # 128
ALIGN = 64             # CAP granularity (128B DMA lines)

# pass-1 sum(x^2): these label slots go to DVE (scalar_tensor_tensor),
# the rest to the scalar engine (Square + accum_out)
SQ_DVE = frozenset({1, 3})
# pass-2 apply: these label slots go to the scalar engine
AP_SC = frozenset({3, 6, 10, 14})

_CACHE = {}


def _build(cap):
    import concourse.bacc as bacc
    import concourse.bass as bass
    from concourse import mybir
    import concourse.tile as tile

    F32 = mybir.dt.float32
    BF16 = mybir.dt.bfloat16
    AF = mybir.ActivationFunctionType
    ALU = mybir.AluOpType
    cols = C * cap

    nc = bacc.Bacc("TRN2", target_bir_lowering=False, debug=False,
                   num_devices=N_CORES)
    xt = nc.dram_tensor("xt", [P, cols], BF16, kind="ExternalInput").ap()
    gbt = nc.dram_tensor("gbt", [P, 3 * C], F32, kind="ExternalInput").ap()
    selrep = nc.dram_tensor("selrep", [P, P], F32, kind="ExternalInput").ap()
    y = nc.dram_tensor("y", [P, cols], BF16, kind="ExternalOutput").ap()

    def x_slice(s):
        return bass.AP(tensor=xt.tensor, offset=s * cap,
                       ap=[[cols, P], [1, cap]])

    def y_slice(s):
        return bass.AP(tensor=y.tensor, offset=s * cap,
                       ap=[[cols, P], [1, cap]])

    with tile.TileContext(nc) as tc:
        with (
            tc.tile_pool(name="const", bufs=1) as const,
            tc.tile_pool(name="xres", bufs=C) as xres,
            tc.tile_pool(name="sqv", bufs=1) as sqv,
            tc.tile_pool(name="sqs", bufs=2) as sqs,
            tc.tile_pool(name="f1p", bufs=2) as f1p,
            tc.tile_pool(name="f2p", bufs=2) as f2p,
            tc.tile_pool(name="ybuf", bufs=4) as ybuf,
            tc.tile_pool(name="ps", bufs=1, space="PSUM") as ps,
        ):
            # ---- constants (small, scalar ring so they land first) ----
            gbt_sb = const.tile([P, 3 * C], F32)
            nc.scalar.dma_start(out=gbt_sb[:], in_=gbt)
            sel_sb = const.tile([P, P], F32)
            nc.scalar.dma_start(out=sel_sb[:], in_=selrep)
            eps_sb = const.tile([P, 1], F32)
            nc.vector.memset(eps_sb[:], EPS)

            # ---- resident x: one [P, cap] tile per label slot ----
            # sync + gpsimd HWDGE/SWDGE rings carry all bulk data; the
            # scalar engine never issues big DMAs (ring-FIFO backpressure
            # would stall its compute stream).
            xts = []
            for s in range(C):
                t = xres.tile([P, cap], BF16, tag="x")
                eng = nc.sync if s % 2 == 0 else nc.gpsimd
                eng.dma_start(out=t[:], in_=x_slice(s))
                xts.append(t)

            # ---- pass 1: raw sums per (partition=(chunk,feat), slot) ----
            # s12 cols [0:C) = sum(x), [C:2C) = sum(x^2)
            # sum(x): two bf16 half-folds at the DVE 2x rate, then a short
            # tensor_reduce (the reduce path itself only runs at 1x).
            h2, h4 = cap // 2, cap // 4
            s12 = const.tile([P, 2 * C], F32)
            for s in range(C):
                f1 = f1p.tile([P, h2], BF16, tag="f1")
                nc.vector.tensor_tensor(out=f1[:], in0=xts[s][:, 0:h2],
                                        in1=xts[s][:, h2:cap], op=ALU.add)
                f2 = f2p.tile([P, h4], BF16, tag="f2")
                nc.vector.tensor_tensor(out=f2[:], in0=f1[:, 0:h4],
                                        in1=f1[:, h4:h2], op=ALU.add)
                nc.vector.tensor_reduce(s12[:, s:s + 1], f2[:],
                                        axis=mybir.AxisListType.X,
                                        op=ALU.add)
                if s in SQ_DVE:
                    scr = sqv.tile([P, cap], BF16, tag="scrv")
                    nc.vector.scalar_tensor_tensor(
                        out=scr[:], in0=xts[s][:], scalar=1.0,
                        in1=xts[s][:], op0=ALU.mult, op1=ALU.mult,
                        accum_out=s12[:, C + s:C + s + 1])
                else:
                    scr = sqs.tile([P, cap], BF16, tag="scrs")
                    nc.scalar.activation(out=scr[:], in_=xts[s][:],
                                         func=AF.Square,
                                         accum_out=s12[:, C + s:C + s + 1])

            # ---- fold chunk partials per feature AND replicate to all
            #      128 partitions in one PE matmul (selrep[p,m]=[p==m mod 16])
            psA = ps.tile([P, 2 * C], F32, tag="psA")
            nc.tensor.matmul(out=psA[:], lhsT=sel_sb[:], rhs=s12[:],
                             start=True, stop=True)

            # ---- stats -> scale/shift on 128 partitions ----
            invn = gbt_sb[:, 2 * C:3 * C]
            meanc = const.tile([P, C], F32)
            nc.vector.tensor_tensor(out=meanc[:], in0=psA[:, 0:C],
                                    in1=invn, op=ALU.mult)
            ex2 = const.tile([P, C], F32)
            nc.vector.tensor_tensor(out=ex2[:], in0=psA[:, C:2 * C],
                                    in1=invn, op=ALU.mult)
            varc = const.tile([P, C], F32)
            nc.vector.tensor_tensor(out=varc[:], in0=meanc[:], in1=meanc[:],
                                    op=ALU.mult)
            nc.vector.tensor_tensor(out=varc[:], in0=ex2[:], in1=varc[:],
                                    op=ALU.subtract)
            stdc = const.tile([P, C], F32)
            nc.scalar.activation(out=stdc[:], in_=varc[:], func=AF.Sqrt,
                                 bias=eps_sb[:])
            istd = const.tile([P, C], F32)
            nc.vector.reciprocal(out=istd[:], in_=stdc[:])
            ss = const.tile([P, 2 * C], F32)
            nc.vector.tensor_tensor(out=ss[:, 0:C], in0=gbt_sb[:, 0:C],
                                    in1=istd[:], op=ALU.mult)
            msc = const.tile([P, C], F32)
            nc.vector.tensor_tensor(out=msc[:], in0=meanc[:],
                                    in1=ss[:, 0:C], op=ALU.mult)
            nc.vector.tensor_tensor(out=ss[:, C:2 * C], in0=gbt_sb[:, C:2 * C],
                                    in1=msc[:], op=ALU.subtract)

            # ---- pass 2: y = x*scale + shift, slot by slot ----
            for s in range(C):
                yb = ybuf.tile([P, cap], BF16, tag="y")
                if s in AP_SC:
                    nc.scalar.activation(out=yb[:], in_=xts[s][:],
                                         func=AF.Identity,
                                         bias=ss[:, C + s:C + s + 1],
                                         scale=ss[:, s:s + 1])
                else:
                    nc.vector.tensor_scalar(out=yb[:], in0=xts[s][:],
                                            scalar1=ss[:, s:s + 1],
                                            scalar2=ss[:, C + s:C + s + 1],
                                            op0=ALU.mult, op1=ALU.add)
                eng = nc.sync if s % 2 == 0 else nc.gpsimd
                eng.dma_start(out=y_slice(s), in_=yb[:])
    nc.finalize()
    return nc


def _get_nc(cap):
    key = ("nc", cap)
    if key not in _CACHE:
        _CACHE[key] = _build(cap)
    return _CACHE[key]


def kernel(x, labels, gamma, beta):
    import ml_dtypes
    from concourse.bass_utils import run_bass_kernel_spmd

    BF = ml_dtypes.bfloat16
    x = np.asarray(x, dtype=np.float32)
    lab = np.asarray(labels).astype(np.int64).ravel()
    gamma = np.asarray(gamma, dtype=np.float32)
    beta = np.asarray(beta, dtype=np.float32)

    counts = np.bincount(lab, minlength=C).astype(np.int64)
    base, rem = counts // NJ, counts % NJ
    ncj = base[None, :] + (np.arange(NJ)[:, None] < rem[None, :])  # [NJ, C]
    cap = int(-(-int(ncj.max()) // ALIGN) * ALIGN)
    cols = C * cap

    order = np.argsort(lab, kind="stable")
    starts = np.zeros(C + 1, np.int64)
    starts[1:] = np.cumsum(counts)
    # col_idx[j, c*cap + t] = original row index (N -> zero/garbage row)
    col_idx = np.full((NJ, cols), N, dtype=np.int64)
    for c in range(C):
        off = starts[c]
        for j in range(NJ):
            m = int(ncj[j, c])
            col_idx[j, c * cap:c * cap + m] = order[off:off + m]
            off += m

    xb = np.concatenate([x.astype(BF), np.zeros((1, F), BF)], axis=0)
    g = xb[col_idx.reshape(-1)]                    # [NJ*cols, F] bf16
    g = g.reshape(NJ, cols, F).transpose(0, 2, 1)  # [NJ, F, cols]

    invn = (1.0 / np.maximum(counts, 1)).astype(np.float32)
    gT, bT = gamma.T, beta.T                       # [F, C]
    selrep = (np.arange(P)[:, None] % FPC == np.arange(P)[None, :] % FPC)
    selrep = np.ascontiguousarray(selrep.astype(np.float32))

    nc = _get_nc(cap)
    in_maps = []
    for k in range(N_CORES):
        f0 = k * FPC
        xk = np.ascontiguousarray(g[:, f0:f0 + FPC, :]).reshape(P, cols)
        gbk = np.concatenate(
            [gT[f0:f0 + FPC], bT[f0:f0 + FPC],
             np.broadcast_to(invn[None, :], (FPC, C))], axis=1,
        ).astype(np.float32)
        gbk = np.ascontiguousarray(np.tile(gbk, (NJ, 1)))  # [P, 3C]
        in_maps.append({
            "xt": xk,
            "gbt": gbk,
            "selrep": selrep,
        })
    res = run_bass_kernel_spmd(nc, in_maps, core_ids=list(range(N_CORES)),
                               **_CACHE.get("run_kwargs", {}))
    _CACHE["last_results"] = res

    ys = np.empty((N + 1, F), dtype=np.float32)    # row N absorbs padding
    for k in range(N_CORES):
        f0 = k * FPC
        yk = np.asarray(res.results[k]["y"]).reshape(NJ, FPC, cols)
        yk = yk.transpose(0, 2, 1).astype(np.float32)  # [NJ, cols, FPC]
        for j in range(NJ):
            ys[col_idx[j], f0:f0 + FPC] = yk[j]
    return np.ascontiguousarray(ys[:N])


# revision 9
# speedup vs baseline: 2.8690x; 1.0261x over previous
"""Conditional BatchNorm1d (training mode) on 8 Trainium2 NeuronCores.

Strategy (feature-parallel, sort-packed, zero collectives):
  - Host: stable-sort rows by label; split each label's rows evenly over 8
    row-chunks; pack a per-core [128, 16*CAP] bf16 array where partition
    p = chunk*16 + i holds feature (16*core + i) of that chunk's rows,
    one fixed-capacity zero-padded column slot per label.
  - Each core owns 16 of the 128 features for ALL 500k rows, so the global
    per-label statistics are computed entirely locally -- no AllReduce.
  - Pass 1 rides mostly on the otherwise-idle PE: per label slot, matmuls
    with a [128,16] chunk-selector lhsT accumulate 248-column groups into
    PSUM (the free-axis reduction becomes PSUM accumulation), then a short
    DVE tensor_reduce collapses [16,248] -> [16,1]. sum(x^2) needs x^2 in
    SBUF: DVE squares 7 slots (2x-rate scalar_tensor_tensor) for the PE,
    the scalar engine handles 9 slots directly via Square + accum_out.
  - Scale/shift are computed on 16 partitions (mirroring the reference
    formulas) and broadcast to 128 partitions with one tiny fp32 matmul.
  - Pass 2: per label slot, y = x*scale + shift as a single per-partition
    tensor_scalar (DVE) or activation Identity (scalar engine), DMA out.
  - x stays SBUF-resident between the passes: HBM traffic is ~16 MB in +
    ~16 MB out per core (bf16). All bulk DMAs ride the sync/gpsimd rings;
    the scalar engine issues none (ring-FIFO backpressure would stall its
    compute stream). gpsimd runs no compute: it shares an SBUF port with
    DVE (exclusive lock) and would stall it.

Everything hardcoded for: x [500000,128] f32, labels [500000] int,
gamma/beta [16,128] f32, 16 conditions. bf16 data path keeps the norm
relative error ~2e-3, well inside the 2e-2 gate.
"""
import numpy as np

N_CORES = 8
N = 500000
F = 128
C = 16
EPS = 1e-5

FPC = F // N_CORES     # 16 features owned per core
NJ = 8                 # row-chunks -> partitions = NJ*FPC = 128
P = NJ * FPC           # 128
WP = 248               # PSUM accumulation width (cap must be a multiple)
ALIGN = 8 * WP         # keeps cap a multiple of WP (and of 64)

# pass-1 sum(x^2): these label slots are squared on DVE and summed on the
# PE; the rest go to the scalar engine (Square + accum_out)
SQ_PE = frozenset(range(0, 7))
# pass-2 apply: these label slots go to the scalar engine
AP_SC = frozenset({5, 10, 14})

_CACHE = {}


def _build(cap):
    import concourse.bacc as bacc
    import concourse.bass as bass
    from concourse import mybir
    import concourse.tile as tile

    F32 = mybir.dt.float32
    BF16 = mybir.dt.bfloat16
    AF = mybir.ActivationFunctionType
    ALU = mybir.AluOpType
    cols = C * cap
    G = cap // WP

    nc = bacc.Bacc("TRN2", target_bir_lowering=False, debug=False,
                   num_devices=N_CORES)
    xt = nc.dram_tensor("xt", [P, cols], BF16, kind="ExternalInput").ap()
    gbt = nc.dram_tensor("gbt", [FPC, 3 * C], F32, kind="ExternalInput").ap()
    selb = nc.dram_tensor("selb", [P, FPC], BF16, kind="ExternalInput").ap()
    self32 = nc.dram_tensor("self32", [P, FPC], F32, kind="ExternalInput").ap()
    rep32 = nc.dram_tensor("rep32", [FPC, P], F32, kind="ExternalInput").ap()
    y = nc.dram_tensor("y", [P, cols], BF16, kind="ExternalOutput").ap()

    def x_slice(s):
        return bass.AP(tensor=xt.tensor, offset=s * cap,
                       ap=[[cols, P], [1, cap]])

    def y_slice(s):
        return bass.AP(tensor=y.tensor, offset=s * cap,
                       ap=[[cols, P], [1, cap]])

    with tile.TileContext(nc) as tc:
        with (
            tc.tile_pool(name="const", bufs=1) as const,
            tc.tile_pool(name="xres", bufs=C) as xres,
            tc.tile_pool(name="sqv", bufs=2) as sqv,
            tc.tile_pool(name="sqs", bufs=2) as sqs,
            tc.tile_pool(name="ybuf", bufs=4) as ybuf,
            tc.tile_pool(name="psx", bufs=3, space="PSUM") as psx,
            tc.tile_pool(name="psq", bufs=2, space="PSUM") as psq,
            tc.tile_pool(name="psg", bufs=1, space="PSUM") as psg,
        ):
            # ---- constants (small, scalar ring so they land first) ----
            gbt_sb = const.tile([FPC, 3 * C], F32)
            nc.scalar.dma_start(out=gbt_sb[:], in_=gbt)
            selb_sb = const.tile([P, FPC], BF16)
            nc.scalar.dma_start(out=selb_sb[:], in_=selb)
            self_sb = const.tile([P, FPC], F32)
            nc.scalar.dma_start(out=self_sb[:], in_=self32)
            rep_sb = const.tile([FPC, P], F32)
            nc.scalar.dma_start(out=rep_sb[:], in_=rep32)
            eps_sb = const.tile([FPC, 1], F32)
            nc.vector.memset(eps_sb[:], EPS)

            # ---- resident x: one [P, cap] tile per label slot ----
            xts = []
            for s in range(C):
                t = xres.tile([P, cap], BF16, tag="x")
                eng = nc.sync if s % 2 == 0 else nc.gpsimd
                eng.dma_start(out=t[:], in_=x_slice(s))
                xts.append(t)

            # ---- pass 1 ----
            # s16[i, 0:C) = global sum(x), [C:2C) = global sum(x^2), both
            # already folded over the 8 chunks (PE contracts partitions).
            s16 = const.tile([FPC, 2 * C], F32)
            s2d = const.tile([P, C], F32)   # scalar-engine raw partials

            def pe_slot_sum(src_ap, out_col):
                ps = (psx if out_col < C else psq).tile(
                    [FPC, WP], F32, tag="acc" if out_col < C else "accq")
                for g in range(G):
                    nc.tensor.matmul(out=ps[:], lhsT=selb_sb[:],
                                     rhs=src_ap[:, g * WP:(g + 1) * WP],
                                     start=(g == 0), stop=(g == G - 1))
                nc.vector.tensor_reduce(s16[:, out_col:out_col + 1], ps[:],
                                        axis=mybir.AxisListType.X, op=ALU.add)

            for s in range(C):
                pe_slot_sum(xts[s], s)                       # sum(x)
                if s in SQ_PE:
                    sq = sqv.tile([P, cap], BF16, tag="scrv")
                    nc.vector.scalar_tensor_tensor(
                        out=sq[:], in0=xts[s][:], scalar=1.0,
                        in1=xts[s][:], op0=ALU.mult, op1=ALU.mult)
                    pe_slot_sum(sq, C + s)                   # sum(x^2)
                else:
                    scr = sqs.tile([P, cap], BF16, tag="scrs")
                    nc.scalar.activation(out=scr[:], in_=xts[s][:],
                                         func=AF.Square,
                                         accum_out=s2d[:, s:s + 1])

            # fold the scalar engine's raw partials over the 8 chunks
            nsc = C - len(SQ_PE)
            s0 = min(s for s in range(C) if s not in SQ_PE)
            psf = psg.tile([FPC, nsc], F32, tag="fold")
            nc.tensor.matmul(out=psf[:], lhsT=self_sb[:],
                             rhs=s2d[:, s0:s0 + nsc], start=True, stop=True)
            nc.vector.tensor_copy(out=s16[:, C + s0:C + s0 + nsc], in_=psf[:])

            # ---- stats -> scale/shift on 16 partitions ----
            invn = gbt_sb[:, 2 * C:3 * C]
            meanc = const.tile([FPC, C], F32)
            nc.vector.tensor_tensor(out=meanc[:], in0=s16[:, 0:C],
                                    in1=invn, op=ALU.mult)
            ex2 = const.tile([FPC, C], F32)
            nc.vector.tensor_tensor(out=ex2[:], in0=s16[:, C:2 * C],
                                    in1=invn, op=ALU.mult)
            varc = const.tile([FPC, C], F32)
            nc.vector.tensor_tensor(out=varc[:], in0=meanc[:], in1=meanc[:],
                                    op=ALU.mult)
            nc.vector.tensor_tensor(out=varc[:], in0=ex2[:], in1=varc[:],
                                    op=ALU.subtract)
            stdc = const.tile([FPC, C], F32)
            nc.scalar.activation(out=stdc[:], in_=varc[:], func=AF.Sqrt,
                                 bias=eps_sb[:])
            istd = const.tile([FPC, C], F32)
            nc.vector.reciprocal(out=istd[:], in_=stdc[:])
            scsh = const.tile([FPC, 2 * C], F32)
            nc.vector.tensor_tensor(out=scsh[:, 0:C], in0=gbt_sb[:, 0:C],
                                    in1=istd[:], op=ALU.mult)
            msc = const.tile([FPC, C], F32)
            nc.vector.tensor_tensor(out=msc[:], in0=meanc[:],
                                    in1=scsh[:, 0:C], op=ALU.mult)
            nc.vector.tensor_tensor(out=scsh[:, C:2 * C], in0=gbt_sb[:, C:2 * C],
                                    in1=msc[:], op=ALU.subtract)

            # broadcast scale/shift to all 128 partitions (tiny fp32 matmul)
            psB = psg.tile([P, 2 * C], F32, tag="rep")
            nc.tensor.matmul(out=psB[:], lhsT=rep_sb[:], rhs=scsh[:],
                             start=True, stop=True)
            ss = const.tile([P, 2 * C], F32)
            nc.vector.tensor_copy(out=ss[:], in_=psB[:])

            # ---- pass 2: y = x*scale + shift, slot by slot ----
            for s in range(C):
                yb = ybuf.tile([P, cap], BF16, tag="y")
                if s in AP_SC:
                    nc.scalar.activation(out=yb[:], in_=xts[s][:],
                                         func=AF.Identity,
                                         bias=ss[:, C + s:C + s + 1],
                                         scale=ss[:, s:s + 1])
                else:
                    nc.vector.tensor_scalar(out=yb[:], in0=xts[s][:],
                                            scalar1=ss[:, s:s + 1],
                                            scalar2=ss[:, C + s:C + s + 1],
                                            op0=ALU.mult, op1=ALU.add)
                eng = nc.sync if s % 2 == 0 else nc.gpsimd
                eng.dma_start(out=y_slice(s), in_=yb[:])
    nc.finalize()
    return nc


def _get_nc(cap):
    key = ("nc", cap)
    if key not in _CACHE:
        _CACHE[key] = _build(cap)
    return _CACHE[key]


def kernel(x, labels, gamma, beta):
    import ml_dtypes
    from concourse.bass_utils import run_bass_kernel_spmd

    BF = ml_dtypes.bfloat16
    x = np.asarray(x, dtype=np.float32)
    lab = np.asarray(labels).astype(np.int64).ravel()
    gamma = np.asarray(gamma, dtype=np.float32)
    beta = np.asarray(beta, dtype=np.float32)

    counts = np.bincount(lab, minlength=C).astype(np.int64)
    base, rem = counts // NJ, counts % NJ
    ncj = base[None, :] + (np.arange(NJ)[:, None] < rem[None, :])  # [NJ, C]
    cap = int(-(-int(ncj.max()) // ALIGN) * ALIGN)
    cols = C * cap

    order = np.argsort(lab, kind="stable")
    starts = np.zeros(C + 1, np.int64)
    starts[1:] = np.cumsum(counts)
    # col_idx[j, c*cap + t] = original row index (N -> zero/garbage row)
    col_idx = np.full((NJ, cols), N, dtype=np.int64)
    for c in range(C):
        off = starts[c]
        for j in range(NJ):
            m = int(ncj[j, c])
            col_idx[j, c * cap:c * cap + m] = order[off:off + m]
            off += m

    xb = np.concatenate([x.astype(BF), np.zeros((1, F), BF)], axis=0)
    g = xb[col_idx.reshape(-1)]                    # [NJ*cols, F] bf16
    g = g.reshape(NJ, cols, F).transpose(0, 2, 1)  # [NJ, F, cols]

    invn = (1.0 / np.maximum(counts, 1)).astype(np.float32)
    gT, bT = gamma.T, beta.T                       # [F, C]
    selm = (np.arange(P)[:, None] % FPC == np.arange(FPC)[None, :])
    selm = np.ascontiguousarray(selm.astype(np.float32))

    nc = _get_nc(cap)
    in_maps = []
    for k in range(N_CORES):
        f0 = k * FPC
        xk = np.ascontiguousarray(g[:, f0:f0 + FPC, :]).reshape(P, cols)
        gbk = np.concatenate(
            [gT[f0:f0 + FPC], bT[f0:f0 + FPC],
             np.broadcast_to(invn[None, :], (FPC, C))], axis=1,
        ).astype(np.float32)
        in_maps.append({
            "xt": xk,
            "gbt": np.ascontiguousarray(gbk),
            "selb": selm.astype(BF),
            "self32": selm,
            "rep32": np.ascontiguousarray(selm.T),
        })
    res = run_bass_kernel_spmd(nc, in_maps, core_ids=list(range(N_CORES)),
                               **_CACHE.get("run_kwargs", {}))
    _CACHE["last_results"] = res

    ys = np.empty((N + 1, F), dtype=np.float32)    # row N absorbs padding
    for k in range(N_CORES):
        f0 = k * FPC
        yk = np.asarray(res.results[k]["y"]).reshape(NJ, FPC, cols)
        yk = yk.transpose(0, 2, 1).astype(np.float32)  # [NJ, cols, FPC]
        for j in range(NJ):
            ys[col_idx[j], f0:f0 + FPC] = yk[j]
    return np.ascontiguousarray(ys[:N])


# revision 10
# speedup vs baseline: 2.9717x; 1.0358x over previous
"""Conditional BatchNorm1d (training mode) on 8 Trainium2 NeuronCores.

Strategy (feature-parallel, sort-packed, zero collectives):
  - Host: stable-sort rows by label; split each label's rows evenly over 8
    row-chunks; pack a per-core [128, 16*CAP] bf16 array where partition
    p = chunk*16 + i holds feature (16*core + i) of that chunk's rows,
    one fixed-capacity zero-padded column slot per label.
  - Each core owns 16 of the 128 features for ALL 500k rows, so the global
    per-label statistics are computed entirely locally -- no AllReduce.
  - Pass 1 rides mostly on the otherwise-idle PE: per label slot, matmuls
    with a [128,16] chunk-selector lhsT accumulate 248-column groups into
    PSUM (the free-axis reduction becomes PSUM accumulation), then a short
    DVE tensor_reduce collapses [16,248] -> [16,1]. sum(x^2) needs x^2 in
    SBUF: DVE squares 7 slots (2x-rate scalar_tensor_tensor) for the PE,
    the scalar engine handles 9 slots directly via Square + accum_out.
  - Scale/shift are computed on 16 partitions (mirroring the reference
    formulas) and broadcast to 128 partitions with one tiny fp32 matmul.
  - Pass 2: per label slot, y = x*scale + shift as a single per-partition
    tensor_scalar (DVE) or activation Identity (scalar engine), DMA out.
  - x stays SBUF-resident between the passes: HBM traffic is ~16 MB in +
    ~16 MB out per core (bf16). All bulk DMAs ride the sync/gpsimd rings;
    the scalar engine issues none (ring-FIFO backpressure would stall its
    compute stream). gpsimd runs no compute: it shares an SBUF port with
    DVE (exclusive lock) and would stall it.

Everything hardcoded for: x [500000,128] f32, labels [500000] int,
gamma/beta [16,128] f32, 16 conditions. bf16 data path keeps the norm
relative error ~2e-3, well inside the 2e-2 gate.
"""
import numpy as np

N_CORES = 8
N = 500000
F = 128
C = 16
EPS = 1e-5

FPC = F // N_CORES     # 16 features owned per core
NJ = 8                 # row-chunks -> partitions = NJ*FPC = 128
P = NJ * FPC           # 128
WP = 496               # PSUM accumulation width (cap must be a multiple)
ALIGN = 8 * WP         # keeps cap a multiple of WP (and of 64)

# pass-1 sum(x^2): these label slots are squared on DVE and summed on the
# PE (they arrive last; the scalar engine would start them too late), the
# rest go to the scalar engine (Square + accum_out)
SQ_PE = frozenset({12, 13, 14, 15})
# pass-2 apply: these label slots go to the scalar engine
AP_SC = frozenset({6, 12})

_CACHE = {}


def _build(cap):
    import concourse.bacc as bacc
    import concourse.bass as bass
    from concourse import mybir
    import concourse.tile as tile

    F32 = mybir.dt.float32
    BF16 = mybir.dt.bfloat16
    AF = mybir.ActivationFunctionType
    ALU = mybir.AluOpType
    cols = C * cap
    G = cap // WP

    nc = bacc.Bacc("TRN2", target_bir_lowering=False, debug=False,
                   num_devices=N_CORES)
    xt = nc.dram_tensor("xt", [P, cols], BF16, kind="ExternalInput").ap()
    gbt = nc.dram_tensor("gbt", [FPC, 3 * C], F32, kind="ExternalInput").ap()
    selb = nc.dram_tensor("selb", [P, FPC], BF16, kind="ExternalInput").ap()
    self32 = nc.dram_tensor("self32", [P, FPC], F32, kind="ExternalInput").ap()
    rep32 = nc.dram_tensor("rep32", [FPC, P], F32, kind="ExternalInput").ap()
    y = nc.dram_tensor("y", [P, cols], BF16, kind="ExternalOutput").ap()

    def x_slice(s):
        return bass.AP(tensor=xt.tensor, offset=s * cap,
                       ap=[[cols, P], [1, cap]])

    def y_slice(s):
        return bass.AP(tensor=y.tensor, offset=s * cap,
                       ap=[[cols, P], [1, cap]])

    with tile.TileContext(nc) as tc:
        with (
            tc.tile_pool(name="const", bufs=1) as const,
            tc.tile_pool(name="xres", bufs=C) as xres,
            tc.tile_pool(name="sqv", bufs=2) as sqv,
            tc.tile_pool(name="sqs", bufs=2) as sqs,
            tc.tile_pool(name="ybuf", bufs=4) as ybuf,
            tc.tile_pool(name="psx", bufs=3, space="PSUM") as psx,
            tc.tile_pool(name="psq", bufs=2, space="PSUM") as psq,
            tc.tile_pool(name="psg", bufs=1, space="PSUM") as psg,
        ):
            # ---- constants (small, scalar ring so they land first) ----
            gbt_sb = const.tile([FPC, 3 * C], F32)
            nc.scalar.dma_start(out=gbt_sb[:], in_=gbt)
            selb_sb = const.tile([P, FPC], BF16)
            nc.scalar.dma_start(out=selb_sb[:], in_=selb)
            self_sb = const.tile([P, FPC], F32)
            nc.scalar.dma_start(out=self_sb[:], in_=self32)
            rep_sb = const.tile([FPC, P], F32)
            nc.scalar.dma_start(out=rep_sb[:], in_=rep32)
            eps_sb = const.tile([FPC, 1], F32)
            nc.vector.memset(eps_sb[:], EPS)

            # ---- resident x: one [P, cap] tile per label slot ----
            xts = []
            for s in range(C):
                t = xres.tile([P, cap], BF16, tag="x")
                eng = nc.sync if s % 2 == 0 else nc.gpsimd
                eng.dma_start(out=t[:], in_=x_slice(s))
                xts.append(t)

            # ---- pass 1 ----
            # s16[i, 0:C) = global sum(x), [C:2C) = global sum(x^2), both
            # already folded over the 8 chunks (PE contracts partitions).
            s16 = const.tile([FPC, 2 * C], F32)
            s2d = const.tile([P, C], F32)   # scalar-engine raw partials

            def pe_slot_sum(src_ap, out_col):
                ps = (psx if out_col < C else psq).tile(
                    [FPC, WP], F32, tag="acc" if out_col < C else "accq")
                for g in range(G):
                    nc.tensor.matmul(out=ps[:], lhsT=selb_sb[:],
                                     rhs=src_ap[:, g * WP:(g + 1) * WP],
                                     start=(g == 0), stop=(g == G - 1))
                nc.vector.tensor_reduce(s16[:, out_col:out_col + 1], ps[:],
                                        axis=mybir.AxisListType.X, op=ALU.add)

            for s in range(C):
                pe_slot_sum(xts[s], s)                       # sum(x)
                if s in SQ_PE:
                    sq = sqv.tile([P, cap], BF16, tag="scrv")
                    nc.vector.tensor_tensor(out=sq[:], in0=xts[s][:],
                                            in1=xts[s][:], op=ALU.mult)
                    pe_slot_sum(sq, C + s)                   # sum(x^2)
                else:
                    scr = sqs.tile([P, cap], BF16, tag="scrs")
                    nc.scalar.activation(out=scr[:], in_=xts[s][:],
                                         func=AF.Square,
                                         accum_out=s2d[:, s:s + 1])

            # fold the scalar engine's raw partials over the 8 chunks
            nsc = C - len(SQ_PE)
            s0 = min(s for s in range(C) if s not in SQ_PE)
            psf = psg.tile([FPC, nsc], F32, tag="fold")
            nc.tensor.matmul(out=psf[:], lhsT=self_sb[:],
                             rhs=s2d[:, s0:s0 + nsc], start=True, stop=True)
            nc.vector.tensor_copy(out=s16[:, C + s0:C + s0 + nsc], in_=psf[:])

            # ---- stats -> scale/shift on 16 partitions ----
            invn = gbt_sb[:, 2 * C:3 * C]
            meanc = const.tile([FPC, C], F32)
            nc.vector.tensor_tensor(out=meanc[:], in0=s16[:, 0:C],
                                    in1=invn, op=ALU.mult)
            ex2 = const.tile([FPC, C], F32)
            nc.vector.tensor_tensor(out=ex2[:], in0=s16[:, C:2 * C],
                                    in1=invn, op=ALU.mult)
            varc = const.tile([FPC, C], F32)
            nc.vector.tensor_tensor(out=varc[:], in0=meanc[:], in1=meanc[:],
                                    op=ALU.mult)
            nc.vector.tensor_tensor(out=varc[:], in0=ex2[:], in1=varc[:],
                                    op=ALU.subtract)
            stdc = const.tile([FPC, C], F32)
            nc.scalar.activation(out=stdc[:], in_=varc[:], func=AF.Sqrt,
                                 bias=eps_sb[:])
            istd = const.tile([FPC, C], F32)
            nc.vector.reciprocal(out=istd[:], in_=stdc[:])
            scsh = const.tile([FPC, 2 * C], F32)
            nc.vector.tensor_tensor(out=scsh[:, 0:C], in0=gbt_sb[:, 0:C],
                                    in1=istd[:], op=ALU.mult)
            msc = const.tile([FPC, C], F32)
            nc.vector.tensor_tensor(out=msc[:], in0=meanc[:],
                                    in1=scsh[:, 0:C], op=ALU.mult)
            nc.vector.tensor_tensor(out=scsh[:, C:2 * C], in0=gbt_sb[:, C:2 * C],
                                    in1=msc[:], op=ALU.subtract)

            # broadcast scale/shift to all 128 partitions (tiny fp32 matmul)
            psB = psg.tile([P, 2 * C], F32, tag="rep")
            nc.tensor.matmul(out=psB[:], lhsT=rep_sb[:], rhs=scsh[:],
                             start=True, stop=True)
            ss = const.tile([P, 2 * C], F32)
            nc.vector.tensor_copy(out=ss[:], in_=psB[:])

            # ---- pass 2: y = x*scale + shift, slot by slot ----
            for s in range(C):
                yb = ybuf.tile([P, cap], BF16, tag="y")
                if s in AP_SC:
                    nc.scalar.activation(out=yb[:], in_=xts[s][:],
                                         func=AF.Identity,
                                         bias=ss[:, C + s:C + s + 1],
                                         scale=ss[:, s:s + 1])
                else:
                    nc.vector.tensor_scalar(out=yb[:], in0=xts[s][:],
                                            scalar1=ss[:, s:s + 1],
                                            scalar2=ss[:, C + s:C + s + 1],
                                            op0=ALU.mult, op1=ALU.add)
                eng = nc.sync if s % 2 == 0 else nc.gpsimd
                eng.dma_start(out=y_slice(s), in_=yb[:])
    nc.finalize()
    return nc


def _get_nc(cap):
    key = ("nc", cap)
    if key not in _CACHE:
        _CACHE[key] = _build(cap)
    return _CACHE[key]


def kernel(x, labels, gamma, beta):
    import ml_dtypes
    from concourse.bass_utils import run_bass_kernel_spmd

    BF = ml_dtypes.bfloat16
    x = np.asarray(x, dtype=np.float32)
    lab = np.asarray(labels).astype(np.int64).ravel()
    gamma = np.asarray(gamma, dtype=np.float32)
    beta = np.asarray(beta, dtype=np.float32)

    counts = np.bincount(lab, minlength=C).astype(np.int64)
    base, rem = counts // NJ, counts % NJ
    ncj = base[None, :] + (np.arange(NJ)[:, None] < rem[None, :])  # [NJ, C]
    cap = int(-(-int(ncj.max()) // ALIGN) * ALIGN)
    cols = C * cap

    order = np.argsort(lab, kind="stable")
    starts = np.zeros(C + 1, np.int64)
    starts[1:] = np.cumsum(counts)
    # col_idx[j, c*cap + t] = original row index (N -> zero/garbage row)
    col_idx = np.full((NJ, cols), N, dtype=np.int64)
    for c in range(C):
        off = starts[c]
        for j in range(NJ):
            m = int(ncj[j, c])
            col_idx[j, c * cap:c * cap + m] = order[off:off + m]
            off += m

    xb = np.concatenate([x.astype(BF), np.zeros((1, F), BF)], axis=0)
    g = xb[col_idx.reshape(-1)]                    # [NJ*cols, F] bf16
    g = g.reshape(NJ, cols, F).transpose(0, 2, 1)  # [NJ, F, cols]

    invn = (1.0 / np.maximum(counts, 1)).astype(np.float32)
    gT, bT = gamma.T, beta.T                       # [F, C]
    selm = (np.arange(P)[:, None] % FPC == np.arange(FPC)[None, :])
    selm = np.ascontiguousarray(selm.astype(np.float32))

    nc = _get_nc(cap)
    in_maps = []
    for k in range(N_CORES):
        f0 = k * FPC
        xk = np.ascontiguousarray(g[:, f0:f0 + FPC, :]).reshape(P, cols)
        gbk = np.concatenate(
            [gT[f0:f0 + FPC], bT[f0:f0 + FPC],
             np.broadcast_to(invn[None, :], (FPC, C))], axis=1,
        ).astype(np.float32)
        in_maps.append({
            "xt": xk,
            "gbt": np.ascontiguousarray(gbk),
            "selb": selm.astype(BF),
            "self32": selm,
            "rep32": np.ascontiguousarray(selm.T),
        })
    res = run_bass_kernel_spmd(nc, in_maps, core_ids=list(range(N_CORES)),
                               **_CACHE.get("run_kwargs", {}))
    _CACHE["last_results"] = res

    ys = np.empty((N + 1, F), dtype=np.float32)    # row N absorbs padding
    for k in range(N_CORES):
        f0 = k * FPC
        yk = np.asarray(res.results[k]["y"]).reshape(NJ, FPC, cols)
        yk = yk.transpose(0, 2, 1).astype(np.float32)  # [NJ, cols, FPC]
        for j in range(NJ):
            ys[col_idx[j], f0:f0 + FPC] = yk[j]
    return np.ascontiguousarray(ys[:N])
